# revision 32
# baseline (speedup 1.0000x reference)
"""C2Q (BiDAF-style) attention kernel for 8 TRN2 NeuronCores.

Pure data parallel: 64 batches sharded 8-per-core. Per batch b (reference):
    S = c @ c_w + (q @ q_w)^T + (c * cq_w) @ q^T + bias      (1024, 128)
    S1 = masked_softmax(S, q_mask, axis=j)
    S2 = masked_softmax(S1, c_mask, axis=i)
    A = S1 @ q ; Bm = S1 @ (S2^T @ c)
    out = [c | A | c*A | c*Bm]                                (1024, 512)

Key algebra: softmax over j is invariant to per-i constants, so the
c @ c_w term and the scalar bias CANCEL in S1 and never need computing.
Only R[j] = q @ q_w + log-mask(q_mask) survives (per-j), and it is a
per-partition bias in the transposed domain.

Device-side formulation (per batch), DMA-lean: c arrives as bf16 twice —
natural order in a host-swizzled layout [p, k*129+d] = [c[k*128+p, d] | 1.0]
(one contiguous 2064B read per partition line) and transposed (cT) for the
S^T matmul. On-device cT derivation was tried and is slower: 64 extra
PE-transpose/DVE-copy round-trips through a 2-buf PSUM pool serialize
worse than the 2MB/core of extra DMA traffic.
    S^T[j,i] = qmodT.T @ cT   (bf16, 2 matmuls of N=512, exp per half)
    E0T      = exp(S^T + R[j])              # ACT bias; stored bf16 [j, 1024]
    per chunk k: E0 = transpose(E0T chunk)  # PE; bf16 psum
                 E0 -> SBUF bf16 with rowsum[i] via DVE accum_out
    rcprow = 1/rowsum ; G = exp(E0*rcprow + cmb[i])   # ACT scale+bias APs
    Traw[j,0:129] = sum_k G_k^T @ [cb_k | 1]  (bf16; col 128 = colsum)
    Ts = Traw * (1/colsum)  -> bf16, next to q in the qq tile
    [Araw|Braw] = E0T_k.T @ [q | Ts]   (bf16, N=256)
    out chunk = [c | Araw*rcprow | c*Araw*rcprow | c*Braw*rcprow]
      c upcast on gpsimd (two chunks per op); A on ACT/DVE alternating;
      [c*A | c*Bm] in one DVE op via a stride-0 broadcast of c.
Output staged two 128-row chunks at a time -> 512KB DMAs.
No max-subtraction needed: |S+R| <= ~30 so exp stays in range.
"""

import os
import numpy as np
import ml_dtypes

import concourse.bass as bass
import concourse.tile as tile
from concourse import bacc, mybir
from concourse.bass_utils import run_bass_kernel_spmd

F32 = mybir.dt.float32
BF16 = mybir.dt.bfloat16
AF = mybir.ActivationFunctionType
ALU = mybir.AluOpType

N_CORES = 8
B, CL, QL, D = 64, 1024, 128, 128
BPC = B // N_CORES          # batches per core
NK = CL // 128              # 128-row chunks per batch
QB = 3 * QL                 # per-batch block in the qq tile: [qmodT | q | Ts]
MASK_NEG = -50.0            # exp(-50+eps) vanishes in f32 sums; in ACT range

LAST_RESULTS = None         # set by kernel() for test.py profiling
SKEW = 1                    # 0: sequential emission; k>0: phase-pipelined
OFF = [0, 1, 2, 3, 4, 4, 4, 4]  # per-phase step offsets within the pipeline


def _build_graph(loop_n=0):
    """loop_n=0: straight-line graph (production). loop_n=N>0: wrap the whole
    computation in a hardware For_i loop repeating it N times (timing only)."""
    nc = bacc.Bacc()

    cb_ext = nc.declare_dram_parameter("cb", [BPC, 128, NK * 129], BF16, isOutput=False)
    cT_ext = nc.declare_dram_parameter("cT", [BPC, D, CL], BF16, isOutput=False)
    qq_ext = nc.declare_dram_parameter("qq", [128, BPC * 2 * QL], BF16, isOutput=False)
    cmR_ext = nc.declare_dram_parameter("cmR", [128, BPC * NK + BPC], F32, isOutput=False)
    id_ext = nc.declare_dram_parameter("ident", [128, 128], BF16, isOutput=False)
    out_ext = nc.declare_dram_parameter("out", [BPC, CL, 4 * D], F32, isOutput=True)

    with tile.TileContext(nc) as tc:
        with (
            tc.tile_pool(name="const", bufs=1) as const,
            tc.tile_pool(name="cbuf", bufs=5) as cbuf,
            tc.tile_pool(name="ctbuf", bufs=4) as ctbuf,
            tc.tile_pool(name="e0tp", bufs=4) as e0tp,
            tc.tile_pool(name="e0p", bufs=12) as e0p,
            tc.tile_pool(name="gp", bufs=3) as gp,
            tc.tile_pool(name="stg", bufs=7) as stg,
            tc.tile_pool(name="rsp", bufs=3) as rsp,
            tc.tile_pool(name="stp", bufs=2, space=bass.MemorySpace.PSUM) as stp,
            tc.tile_pool(name="tpp", bufs=2, space=bass.MemorySpace.PSUM) as tpp,
            tc.tile_pool(name="trawp", bufs=1, space=bass.MemorySpace.PSUM) as trawp,
            tc.tile_pool(name="abp", bufs=3, space=bass.MemorySpace.PSUM) as abp,
        ):
            ident = const.tile([128, 128], BF16, tag="ident")
            nc.sync.dma_start(ident[:], id_ext[:])
            cmR = const.tile([128, BPC * NK + BPC], F32, tag="cmR")
            nc.sync.dma_start(cmR[:], cmR_ext[:])
            # all 8 batches' [qmodT | q] prefetched in one DMA; Ts filled later
            qq_all = const.tile([128, BPC * QB], BF16, tag="qq_all")
            nc.sync.dma_start(
                qq_all[:, 0:2 * QL],
                qq_ext[:, 0:2 * QL],
            )

            def _ph0(b, s):
                # input DMAs
                s["qmod"] = qq_all[:, b * QB:b * QB + QL]
                s["qts"] = qq_all[:, b * QB + QL:b * QB + QB]  # [q | Ts]
                s["ts_dst"] = qq_all[:, b * QB + 2 * QL:b * QB + QB]
                # cb: 8 groups of [128 ctx cols | ones col], host-swizzled so
                # each partition line is contiguous in DRAM
                cb_t = cbuf.tile([128, NK * 129], BF16, tag="cb")
                nc.sync.dma_start(cb_t[:], cb_ext[b])
                cT_t = ctbuf.tile([128, CL], BF16, tag="cT")
                nc.sync.dma_start(cT_t[:], cT_ext[b])
                if b == 0:
                    nc.sync.dma_start(
                        qq_all[:].rearrange("p (b t) -> p b t", t=QB)[:, 1:, 0:2 * QL],
                        qq_ext[:].rearrange("p (b t) -> p b t", t=2 * QL)[:, 1:, :],
                    )
                s["cb_t"] = cb_t
                s["cT_t"] = cT_t

            def _ph1(b, s):
                # S^T = qmodT.T @ cT ; E0T = exp(S^T + R[j])  [j, 1024] bf16
                s["rowsum"] = rsp.tile([128, NK], F32, tag="rowsum", name="rowsum")
                s["rcprow"] = rsp.tile([128, NK], F32, tag="rcprow", name="rcprow")
                s["rcp2"] = rsp.tile([128, 1], F32, tag="rcp2", name="rcp2")
                e0t_t = e0tp.tile([128, CL], BF16, tag="e0t")
                for h in range(2):
                    sp = stp.tile([128, 512], F32, tag="sp")
                    nc.tensor.matmul(
                        sp[:], s["qmod"], s["cT_t"][:, h * 512:(h + 1) * 512]
                    )
                    nc.scalar.activation(
                        e0t_t[:, h * 512:(h + 1) * 512], sp[:], AF.Exp,
                        bias=cmR[:, BPC * NK + b:BPC * NK + b + 1],
                    )
                s["e0t_t"] = e0t_t

            def _ph2(b, s):
                # per chunk: E0 natural (bf16) + rowsum via DVE accum
                e0_l = []
                for k in range(NK):
                    ep = tpp.tile([128, 128], BF16, tag="ep")
                    nc.tensor.transpose(
                        ep[:], s["e0t_t"][:, k * 128:(k + 1) * 128], ident[:]
                    )
                    e0_t = e0p.tile([128, 128], BF16, tag="e0")
                    nc.vector.tensor_scalar(
                        e0_t[:], ep[:], 1.0, 0.0, ALU.mult, ALU.add,
                        accum_out=s["rowsum"][:, k:k + 1],
                    )
                    e0_l.append(e0_t)
                nc.vector.reciprocal(s["rcprow"][:], s["rowsum"][:])
                s["e0_l"] = e0_l

            def _ph3(b, s):
                # G = exp(E0*rcprow + cmb) ; Traw accum (col 128 = colsum)
                traw = trawp.tile([128, 129], F32, tag="traw")
                for k in range(NK):
                    col = b * NK + k
                    g_t = gp.tile([128, QL], BF16, tag="g")
                    nc.scalar.activation(
                        g_t[:], s["e0_l"][k][:], AF.Exp,
                        bias=cmR[:, col:col + 1],
                        scale=s["rcprow"][:, k:k + 1],
                    )
                    nc.tensor.matmul(
                        traw[:, 0:129], g_t[:],
                        s["cb_t"][:, k * 129:(k + 1) * 129],
                        start=(k == 0), stop=(k == NK - 1),
                    )
                nc.vector.reciprocal(s["rcp2"][:], traw[:, 128:129])
                nc.scalar.activation(s["ts_dst"], traw[:, 0:128], AF.Copy, scale=s["rcp2"][:])

            def _ph4(b, s, k2):
                # [Araw|Braw] = E0T_k.T @ [q | Ts] ; stage 2 chunks per DMA
                rcprow = s["rcprow"]
                outg = out_ext[b].rearrange("(g p) d -> p g d", p=128)
                cbg = s["cb_t"][:].rearrange("p (k d) -> p k d", d=129)
                if True:
                    st = stg.tile([128, 8 * D], F32, tag="st")
                    stg4 = st[:].rearrange("p (g d) -> p g d", d=128)
                    # c cols of both chunks in one gpsimd op (bf16 -> f32)
                    nc.gpsimd.tensor_copy(
                        st[:].rearrange("p (g d) -> p g d", d=512)[:, :, 0:128],
                        cbg[:, 2 * k2:2 * k2 + 2, 0:128],
                    )
                    for j in range(2):
                        k = 2 * k2 + j
                        off = j * 4 * D
                        ab = abp.tile([128, 2 * QL], F32, tag="ab")
                        nc.tensor.matmul(
                            ab[:], s["e0t_t"][:, k * 128:(k + 1) * 128], s["qts"]
                        )
                        # col 128:256 = A = Araw * rcprow (ACT/DVE alternate
                        # by chunk to balance engine load)
                        if k % 2 == 0:
                            nc.scalar.activation(
                                st[:, off + 128:off + 256], ab[:, 0:128],
                                AF.Copy, scale=rcprow[:, k:k + 1],
                            )
                        else:
                            nc.vector.tensor_scalar_mul(
                                st[:, off + 128:off + 256], ab[:, 0:128],
                                rcprow[:, k:k + 1],
                            )
                        # cols 256:512 = [c*A | c*Bm] in one DVE op:
                        # (ab[A|B] * rcprow) * [c | c]  (stride-0 broadcast c)
                        ab2 = ab[:].rearrange("p (g d) -> p g d", d=128)
                        out2 = stg4[:, 4 * j + 2:4 * j + 4, :]
                        cc1 = cbg[:, k:k + 1, 0:128]
                        cc2, _ = bass.broadcast_tensor_aps(cc1, ab2)
                        nc.vector.scalar_tensor_tensor(
                            out2, ab2, rcprow[:, k:k + 1],
                            cc2, ALU.mult, ALU.mult,
                        )
                    nc.sync.dma_start(outg[:, 2 * k2:2 * k2 + 2, :], st[:])

            def _mk4(k2):
                return lambda b, s: _ph4(b, s, k2)

            PHASES = [_ph0, _ph1, _ph2, _ph3] + [_mk4(k2) for k2 in range(4)]

            def _emit_all():
                states = [dict() for _ in range(BPC)]
                if SKEW == 0:
                    for b in range(BPC):
                        for ph in PHASES:
                            ph(b, states[b])
                else:
                    # software-pipelined: batch b runs phase p at step
                    # b*SKEW + OFF[p]
                    last = (BPC - 1) * SKEW + max(OFF)
                    for step in range(last + 1):
                        for b in range(BPC):
                            for p in range(len(PHASES)):
                                if b * SKEW + OFF[p] == step:
                                    PHASES[p](b, states[b])

            if loop_n:
                with tc.For_i(0, loop_n, 1):
                    _emit_all()
            else:
                _emit_all()
    return nc


def _prep(c, q, c_mask, q_mask, c_weight, q_weight, cq_weight, bias):
    c = np.asarray(c, dtype=np.float32)
    q = np.asarray(q, dtype=np.float32)
    c_mask = np.asarray(c_mask)
    q_mask = np.asarray(q_mask)
    q_weight = np.asarray(q_weight, dtype=np.float32)
    cq_weight = np.asarray(cq_weight, dtype=np.float32)

    # host-side prep (tiny). NOTE: c@c_weight and bias cancel in softmax_j.
    s1 = (q.reshape(-1, D) @ q_weight).reshape(B, QL)          # (B, 128)
    R = s1 + np.where(q_mask > 0, 0.0, MASK_NEG).astype(np.float32)
    cmb = np.where(c_mask > 0, 0.0, MASK_NEG).astype(np.float32)  # (B, 1024)

    # cb swizzle: [B, p, k*129 + d] = [c[b, k*128+p, d] | 1.0], bf16
    cb = np.ones((B, 128, NK, 129), dtype=ml_dtypes.bfloat16)
    cb[:, :, :, 0:128] = (
        c.reshape(B, NK, 128, D).transpose(0, 2, 1, 3).astype(ml_dtypes.bfloat16)
    )
    cb = cb.reshape(B, 128, NK * 129)

    cT = np.ascontiguousarray(c.transpose(0, 2, 1)).astype(ml_dtypes.bfloat16)
    qmodT = np.ascontiguousarray(
        (q * cq_weight.reshape(1, 1, D)).transpose(0, 2, 1)
    ).astype(ml_dtypes.bfloat16)
    qT_rows = q.astype(ml_dtypes.bfloat16)                     # (B, 128, 128) [j, e]
    qq = np.concatenate([qmodT, qT_rows], axis=2)              # (B, 128, 256)

    in_maps = []
    for core in range(N_CORES):
        sl = slice(core * BPC, (core + 1) * BPC)
        cmT = cmb[sl].reshape(BPC, NK, 128).transpose(2, 0, 1).reshape(128, BPC * NK)
        cmR = np.ascontiguousarray(
            np.concatenate([cmT, R[sl].T], axis=1)             # (128, 64+8)
        )
        qq_core = np.ascontiguousarray(
            qq[sl].transpose(1, 0, 2).reshape(128, BPC * 2 * QL)
        )
        in_maps.append({
            "cb": np.ascontiguousarray(cb[sl]),
            "cT": cT[sl],
            "qq": qq_core,
            "cmR": cmR,
            "ident": np.eye(128, dtype=ml_dtypes.bfloat16),
        })
    return in_maps


def make_in_maps():
    """For the local test/compare harness only (imports reference)."""
    import reference
    inputs = {k: np.asarray(v) for k, v in reference.setup_inputs().items()}
    return _prep(**inputs)


def kernel(c, q, c_mask, q_mask, c_weight, q_weight, cq_weight, bias):
    global LAST_RESULTS
    in_maps = _prep(c, q, c_mask, q_mask, c_weight, q_weight, cq_weight, bias)
    os.environ["BASS_NEVER_TRACE"] = "1"  # no NTFF hook in this container
    nc = _build_graph()
    nc.finalize()
    res = run_bass_kernel_spmd(nc, in_maps, core_ids=list(range(N_CORES)))
    LAST_RESULTS = (nc, in_maps)
    return np.concatenate([res.results[i]["out"] for i in range(N_CORES)], axis=0)


# revision 33
# speedup vs baseline: 1.1644x; 1.1644x over previous
"""C2Q (BiDAF-style) attention kernel for 8 TRN2 NeuronCores.

Pure data parallel: 64 batches sharded 8-per-core. Per batch b (reference):
    S = c @ c_w + (q @ q_w)^T + (c * cq_w) @ q^T + bias      (1024, 128)
    S1 = masked_softmax(S, q_mask, axis=j)
    S2 = masked_softmax(S1, c_mask, axis=i)
    A = S1 @ q ; Bm = S1 @ (S2^T @ c)
    out = [c | A | c*A | c*Bm]                                (1024, 512)

Key algebra: softmax over j is invariant to per-i constants, so the
c @ c_w term and the scalar bias CANCEL in S1 and never need computing.
Only R[j] = q @ q_w + log-mask(q_mask) survives (per-j), and it is a
per-partition bias in the transposed domain.

Device-side formulation (per batch), DMA-lean: c arrives as bf16 twice —
natural order in a host-swizzled layout [p, k*129+d] = [c[k*128+p, d] | 1.0]
(one contiguous 2064B read per partition line) and transposed (cT) for the
S^T matmul. On-device cT derivation was tried and is slower: 64 extra
PE-transpose/DVE-copy round-trips through a 2-buf PSUM pool serialize
worse than the 2MB/core of extra DMA traffic.
    S^T[j,i] = qmodT.T @ cT   (bf16, 2 matmuls of N=512, exp per half)
    E0T      = exp(S^T + R[j])              # ACT bias; stored bf16 [j, 1024]
    per chunk k: E0 = transpose(E0T chunk)  # PE; bf16 psum
                 E0 -> SBUF bf16 with rowsum[i] via DVE accum_out
    rcprow = 1/rowsum ; G = exp(E0*rcprow + cmb[i])   # ACT scale+bias APs
    Traw[j,0:129] = sum_k G_k^T @ [cb_k | 1]  (bf16; col 128 = colsum)
    Ts = Traw * (1/colsum)  -> bf16, next to q in the qq tile
    [Araw|Braw] = E0T_k.T @ [q | Ts]   (bf16, N=256)
    out chunk = [c | Araw*rcprow | c*Araw*rcprow | c*Braw*rcprow]
      c upcast on gpsimd (two chunks per op); A on ACT/DVE alternating;
      [c*A | c*Bm] in one DVE op via a stride-0 broadcast of c.
Output staged two 128-row chunks at a time -> 512KB DMAs.
No max-subtraction needed: |S+R| <= ~30 so exp stays in range.
"""

import os
import numpy as np
import ml_dtypes

import concourse.bass as bass
import concourse.tile as tile
from concourse import bacc, mybir
from concourse.bass_utils import run_bass_kernel_spmd

F32 = mybir.dt.float32
BF16 = mybir.dt.bfloat16
AF = mybir.ActivationFunctionType
ALU = mybir.AluOpType

N_CORES = 8
B, CL, QL, D = 64, 1024, 128, 128
BPC = B // N_CORES          # batches per core
NK = CL // 128              # 128-row chunks per batch
QB = 3 * QL                 # per-batch block in the qq tile: [qmodT | q | Ts]
MASK_NEG = -50.0            # exp(-50+eps) vanishes in f32 sums; in ACT range

LAST_RESULTS = None         # set by kernel() for test.py profiling
SKEW = 1                    # 0: sequential emission; k>0: phase-pipelined
OFF = [0, 1, 2, 3, 4, 4, 4, 4]  # per-phase step offsets within the pipeline


def _build_graph(loop_n=0):
    """loop_n=0: straight-line graph (production). loop_n=N>0: wrap the whole
    computation in a hardware For_i loop repeating it N times (timing only)."""
    nc = bacc.Bacc()

    cb_ext = nc.declare_dram_parameter("cb", [BPC, 128, NK * 129], BF16, isOutput=False)
    cT_ext = nc.declare_dram_parameter("cT", [BPC, D, CL], BF16, isOutput=False)
    qq_ext = nc.declare_dram_parameter("qq", [128, BPC * 2 * QL], BF16, isOutput=False)
    cmR_ext = nc.declare_dram_parameter("cmR", [128, BPC * NK + BPC], F32, isOutput=False)
    id_ext = nc.declare_dram_parameter("ident", [128, 128], BF16, isOutput=False)
    out_ext = nc.declare_dram_parameter("out", [BPC, CL, 4 * D], F32, isOutput=True)

    with tile.TileContext(nc) as tc:
        with (
            tc.tile_pool(name="const", bufs=1) as const,
            tc.tile_pool(name="cbuf", bufs=5) as cbuf,
            tc.tile_pool(name="ctbuf", bufs=4) as ctbuf,
            tc.tile_pool(name="e0tp", bufs=4) as e0tp,
            tc.tile_pool(name="e0p", bufs=12) as e0p,
            tc.tile_pool(name="gp", bufs=3) as gp,
            tc.tile_pool(name="stg", bufs=7) as stg,
            tc.tile_pool(name="rsp", bufs=3) as rsp,
            tc.tile_pool(name="stp", bufs=2, space=bass.MemorySpace.PSUM) as stp,
            tc.tile_pool(name="tpp", bufs=2, space=bass.MemorySpace.PSUM) as tpp,
            tc.tile_pool(name="trawp", bufs=1, space=bass.MemorySpace.PSUM) as trawp,
            tc.tile_pool(name="abp", bufs=3, space=bass.MemorySpace.PSUM) as abp,
        ):
            # batch 0's critical-path inputs go first: qq block 0, then
            # (inside ph0(0)) cb0/cT0; ident/cmR defer behind them
            qq_all = const.tile([128, BPC * QB], BF16, tag="qq_all")
            nc.sync.dma_start(
                qq_all[:, 0:2 * QL],
                qq_ext[:, 0:2 * QL],
            )
            ident = const.tile([128, 128], BF16, tag="ident")
            cmR = const.tile([128, BPC * NK + BPC], F32, tag="cmR")

            def _ph0(b, s):
                # input DMAs
                s["qmod"] = qq_all[:, b * QB:b * QB + QL]
                s["qts"] = qq_all[:, b * QB + QL:b * QB + QB]  # [q | Ts]
                s["ts_dst"] = qq_all[:, b * QB + 2 * QL:b * QB + QB]
                # cb: 8 groups of [128 ctx cols | ones col], host-swizzled so
                # each partition line is contiguous in DRAM
                cb_t = cbuf.tile([128, NK * 129], BF16, tag="cb")
                nc.sync.dma_start(cb_t[:], cb_ext[b])
                cT_t = ctbuf.tile([128, CL], BF16, tag="cT")
                nc.sync.dma_start(cT_t[:], cT_ext[b])
                if b == 0:
                    nc.sync.dma_start(cmR[:], cmR_ext[:])
                    nc.sync.dma_start(ident[:], id_ext[:])
                    nc.sync.dma_start(
                        qq_all[:].rearrange("p (b t) -> p b t", t=QB)[:, 1:, 0:2 * QL],
                        qq_ext[:].rearrange("p (b t) -> p b t", t=2 * QL)[:, 1:, :],
                    )
                s["cb_t"] = cb_t
                s["cT_t"] = cT_t

            def _ph1(b, s):
                # S^T = qmodT.T @ cT ; E0T = exp(S^T + R[j])  [j, 1024] bf16
                s["rowsum"] = rsp.tile([128, NK], F32, tag="rowsum", name="rowsum")
                s["rcprow"] = rsp.tile([128, NK], F32, tag="rcprow", name="rcprow")
                s["rcp2"] = rsp.tile([128, 1], F32, tag="rcp2", name="rcp2")
                e0t_t = e0tp.tile([128, CL], BF16, tag="e0t")
                for h in range(2):
                    sp = stp.tile([128, 512], F32, tag="sp")
                    nc.tensor.matmul(
                        sp[:], s["qmod"], s["cT_t"][:, h * 512:(h + 1) * 512]
                    )
                    nc.scalar.activation(
                        e0t_t[:, h * 512:(h + 1) * 512], sp[:], AF.Exp,
                        bias=cmR[:, BPC * NK + b:BPC * NK + b + 1],
                    )
                s["e0t_t"] = e0t_t

            def _ph2(b, s):
                # per chunk: E0 natural (bf16) + rowsum via DVE accum
                e0_l = []
                for k in range(NK):
                    ep = tpp.tile([128, 128], BF16, tag="ep")
                    nc.tensor.transpose(
                        ep[:], s["e0t_t"][:, k * 128:(k + 1) * 128], ident[:]
                    )
                    e0_t = e0p.tile([128, 128], BF16, tag="e0")
                    nc.vector.tensor_scalar(
                        e0_t[:], ep[:], 1.0, 0.0, ALU.mult, ALU.add,
                        accum_out=s["rowsum"][:, k:k + 1],
                    )
                    e0_l.append(e0_t)
                nc.vector.reciprocal(s["rcprow"][:], s["rowsum"][:])
                s["e0_l"] = e0_l

            def _ph3(b, s):
                # G = exp(E0*rcprow + cmb) ; Traw accum (col 128 = colsum)
                traw = trawp.tile([128, 129], F32, tag="traw")
                for k in range(NK):
                    col = b * NK + k
                    g_t = gp.tile([128, QL], BF16, tag="g")
                    nc.scalar.activation(
                        g_t[:], s["e0_l"][k][:], AF.Exp,
                        bias=cmR[:, col:col + 1],
                        scale=s["rcprow"][:, k:k + 1],
                    )
                    nc.tensor.matmul(
                        traw[:, 0:129], g_t[:],
                        s["cb_t"][:, k * 129:(k + 1) * 129],
                        start=(k == 0), stop=(k == NK - 1),
                    )
                nc.vector.reciprocal(s["rcp2"][:], traw[:, 128:129])
                nc.scalar.activation(s["ts_dst"], traw[:, 0:128], AF.Copy, scale=s["rcp2"][:])

            def _ph4(b, s, k2):
                # [Araw|Braw] = E0T_k.T @ [q | Ts] ; stage 2 chunks per DMA
                rcprow = s["rcprow"]
                outg = out_ext[b].rearrange("(g p) d -> p g d", p=128)
                cbg = s["cb_t"][:].rearrange("p (k d) -> p k d", d=129)
                if True:
                    st = stg.tile([128, 8 * D], F32, tag="st")
                    stg4 = st[:].rearrange("p (g d) -> p g d", d=128)
                    # c cols of both chunks in one gpsimd op (bf16 -> f32)
                    nc.gpsimd.tensor_copy(
                        st[:].rearrange("p (g d) -> p g d", d=512)[:, :, 0:128],
                        cbg[:, 2 * k2:2 * k2 + 2, 0:128],
                    )
                    for j in range(2):
                        k = 2 * k2 + j
                        off = j * 4 * D
                        ab = abp.tile([128, 2 * QL], F32, tag="ab")
                        nc.tensor.matmul(
                            ab[:], s["e0t_t"][:, k * 128:(k + 1) * 128], s["qts"]
                        )
                        # col 128:256 = A = Araw * rcprow (ACT/DVE alternate
                        # by chunk to balance engine load)
                        if k % 2 == 0:
                            nc.scalar.activation(
                                st[:, off + 128:off + 256], ab[:, 0:128],
                                AF.Copy, scale=rcprow[:, k:k + 1],
                            )
                        else:
                            nc.vector.tensor_scalar_mul(
                                st[:, off + 128:off + 256], ab[:, 0:128],
                                rcprow[:, k:k + 1],
                            )
                        # cols 256:512 = [c*A | c*Bm] in one DVE op:
                        # (ab[A|B] * rcprow) * [c | c]  (stride-0 broadcast c)
                        ab2 = ab[:].rearrange("p (g d) -> p g d", d=128)
                        out2 = stg4[:, 4 * j + 2:4 * j + 4, :]
                        cc1 = cbg[:, k:k + 1, 0:128]
                        cc2, _ = bass.broadcast_tensor_aps(cc1, ab2)
                        nc.vector.scalar_tensor_tensor(
                            out2, ab2, rcprow[:, k:k + 1],
                            cc2, ALU.mult, ALU.mult,
                        )
                    nc.sync.dma_start(outg[:, 2 * k2:2 * k2 + 2, :], st[:])

            def _mk4(k2):
                return lambda b, s: _ph4(b, s, k2)

            PHASES = [_ph0, _ph1, _ph2, _ph3] + [_mk4(k2) for k2 in range(4)]

            def _emit_all():
                states = [dict() for _ in range(BPC)]
                if SKEW == 0:
                    for b in range(BPC):
                        for ph in PHASES:
                            ph(b, states[b])
                else:
                    # software-pipelined: batch b runs phase p at step
                    # b*SKEW + OFF[p]
                    last = (BPC - 1) * SKEW + max(OFF)
                    for step in range(last + 1):
                        for b in range(BPC):
                            for p in range(len(PHASES)):
                                if b * SKEW + OFF[p] == step:
                                    PHASES[p](b, states[b])

            if loop_n:
                with tc.For_i(0, loop_n, 1):
                    _emit_all()
            else:
                _emit_all()
    return nc


def _prep(c, q, c_mask, q_mask, c_weight, q_weight, cq_weight, bias):
    c = np.asarray(c, dtype=np.float32)
    q = np.asarray(q, dtype=np.float32)
    c_mask = np.asarray(c_mask)
    q_mask = np.asarray(q_mask)
    q_weight = np.asarray(q_weight, dtype=np.float32)
    cq_weight = np.asarray(cq_weight, dtype=np.float32)

    # host-side prep (tiny). NOTE: c@c_weight and bias cancel in softmax_j.
    s1 = (q.reshape(-1, D) @ q_weight).reshape(B, QL)          # (B, 128)
    R = s1 + np.where(q_mask > 0, 0.0, MASK_NEG).astype(np.float32)
    cmb = np.where(c_mask > 0, 0.0, MASK_NEG).astype(np.float32)  # (B, 1024)

    # cb swizzle: [B, p, k*129 + d] = [c[b, k*128+p, d] | 1.0], bf16
    cb = np.ones((B, 128, NK, 129), dtype=ml_dtypes.bfloat16)
    cb[:, :, :, 0:128] = (
        c.reshape(B, NK, 128, D).transpose(0, 2, 1, 3).astype(ml_dtypes.bfloat16)
    )
    cb = cb.reshape(B, 128, NK * 129)

    cT = np.ascontiguousarray(c.transpose(0, 2, 1)).astype(ml_dtypes.bfloat16)
    qmodT = np.ascontiguousarray(
        (q * cq_weight.reshape(1, 1, D)).transpose(0, 2, 1)
    ).astype(ml_dtypes.bfloat16)
    qT_rows = q.astype(ml_dtypes.bfloat16)                     # (B, 128, 128) [j, e]
    qq = np.concatenate([qmodT, qT_rows], axis=2)              # (B, 128, 256)

    in_maps = []
    for core in range(N_CORES):
        sl = slice(core * BPC, (core + 1) * BPC)
        cmT = cmb[sl].reshape(BPC, NK, 128).transpose(2, 0, 1).reshape(128, BPC * NK)
        cmR = np.ascontiguousarray(
            np.concatenate([cmT, R[sl].T], axis=1)             # (128, 64+8)
        )
        qq_core = np.ascontiguousarray(
            qq[sl].transpose(1, 0, 2).reshape(128, BPC * 2 * QL)
        )
        in_maps.append({
            "cb": np.ascontiguousarray(cb[sl]),
            "cT": cT[sl],
            "qq": qq_core,
            "cmR": cmR,
            "ident": np.eye(128, dtype=ml_dtypes.bfloat16),
        })
    return in_maps


def make_in_maps():
    """For the local test/compare harness only (imports reference)."""
    import reference
    inputs = {k: np.asarray(v) for k, v in reference.setup_inputs().items()}
    return _prep(**inputs)


def kernel(c, q, c_mask, q_mask, c_weight, q_weight, cq_weight, bias):
    global LAST_RESULTS
    in_maps = _prep(c, q, c_mask, q_mask, c_weight, q_weight, cq_weight, bias)
    os.environ["BASS_NEVER_TRACE"] = "1"  # no NTFF hook in this container
    nc = _build_graph()
    nc.finalize()
    res = run_bass_kernel_spmd(nc, in_maps, core_ids=list(range(N_CORES)))
    LAST_RESULTS = (nc, in_maps)
    return np.concatenate([res.results[i]["out"] for i in range(N_CORES)], axis=0)


# revision 34
# speedup vs baseline: 1.2805x; 1.0998x over previous
"""C2Q (BiDAF-style) attention kernel for 8 TRN2 NeuronCores.

Pure data parallel: 64 batches sharded 8-per-core. Per batch b (reference):
    S = c @ c_w + (q @ q_w)^T + (c * cq_w) @ q^T + bias      (1024, 128)
    S1 = masked_softmax(S, q_mask, axis=j)
    S2 = masked_softmax(S1, c_mask, axis=i)
    A = S1 @ q ; Bm = S1 @ (S2^T @ c)
    out = [c | A | c*A | c*Bm]                                (1024, 512)

Key algebra: softmax over j is invariant to per-i constants, so the
c @ c_w term and the scalar bias CANCEL in S1 and never need computing.
Only R[j] = q @ q_w + log-mask(q_mask) survives (per-j), and it is a
per-partition bias in the transposed domain.

Device-side formulation (per batch), DMA-lean: c arrives as bf16 twice —
natural order in a host-swizzled layout [p, k*129+d] = [c[k*128+p, d] | 1.0]
(one contiguous 2064B read per partition line) and transposed (cT) for the
S^T matmul. On-device cT derivation was tried and is slower: 64 extra
PE-transpose/DVE-copy round-trips through a 2-buf PSUM pool serialize
worse than the 2MB/core of extra DMA traffic.
    S^T[j,i] = qmodT.T @ cT   (bf16, 2 matmuls of N=512, exp per half)
    E0T      = exp(S^T + R[j])              # ACT bias; stored bf16 [j, 1024]
    per chunk k: E0 = transpose(E0T chunk)  # PE; bf16 psum
                 E0 -> SBUF bf16 with rowsum[i] via DVE accum_out
    rcprow = 1/rowsum ; G = exp(E0*rcprow + cmb[i])   # ACT scale+bias APs
    Traw[j,0:129] = sum_k G_k^T @ [cb_k | 1]  (bf16; col 128 = colsum)
    Ts = Traw * (1/colsum)  -> bf16, next to q in the qq tile
    [Araw|Braw] = E0T_k.T @ [q | Ts]   (bf16, N=256)
    out chunk = [c | Araw*rcprow | c*Araw*rcprow | c*Braw*rcprow]
      c upcast on gpsimd (two chunks per op); A on ACT/DVE alternating;
      [c*A | c*Bm] in one DVE op via a stride-0 broadcast of c.
Output staged two 128-row chunks at a time -> 512KB DMAs.
No max-subtraction needed: |S+R| <= ~30 so exp stays in range.
"""

import os
import numpy as np
import ml_dtypes

import concourse.bass as bass
import concourse.tile as tile
from concourse import bacc, mybir
from concourse.bass_utils import run_bass_kernel_spmd

F32 = mybir.dt.float32
BF16 = mybir.dt.bfloat16
AF = mybir.ActivationFunctionType
ALU = mybir.AluOpType

N_CORES = 8
B, CL, QL, D = 64, 1024, 128, 128
BPC = B // N_CORES          # batches per core
NK = CL // 128              # 128-row chunks per batch
QB = 3 * QL                 # per-batch block in the qq tile: [qmodT | q | Ts]
MASK_NEG = -50.0            # exp(-50+eps) vanishes in f32 sums; in ACT range

LAST_RESULTS = None         # set by kernel() for test.py profiling
SKEW = 1                    # 0: sequential emission; k>0: phase-pipelined
OFF = [0, 1, 2, 3, 4, 4, 4, 4]  # per-phase step offsets within the pipeline


def _build_graph(loop_n=0):
    """loop_n=0: straight-line graph (production). loop_n=N>0: wrap the whole
    computation in a hardware For_i loop repeating it N times (timing only)."""
    nc = bacc.Bacc()

    cb_ext = nc.declare_dram_parameter("cb", [BPC, 128, NK * 129], BF16, isOutput=False)
    cT_ext = nc.declare_dram_parameter("cT", [BPC, D, CL], BF16, isOutput=False)
    qq_ext = nc.declare_dram_parameter("qq", [128, BPC * 2 * QL], BF16, isOutput=False)
    cmR_ext = nc.declare_dram_parameter("cmR", [128, BPC * NK + BPC], F32, isOutput=False)
    id_ext = nc.declare_dram_parameter("ident", [128, 128], BF16, isOutput=False)
    out_ext = nc.declare_dram_parameter("out", [BPC, CL, 4 * D], F32, isOutput=True)

    with tile.TileContext(nc) as tc:
        with (
            tc.tile_pool(name="const", bufs=1) as const,
            tc.tile_pool(name="cbuf", bufs=5) as cbuf,
            tc.tile_pool(name="ctbuf", bufs=4) as ctbuf,
            tc.tile_pool(name="e0tp", bufs=4) as e0tp,
            tc.tile_pool(name="e0p", bufs=12) as e0p,
            tc.tile_pool(name="gp", bufs=3) as gp,
            tc.tile_pool(name="stg", bufs=7) as stg,
            tc.tile_pool(name="rsp", bufs=3) as rsp,
            tc.tile_pool(name="stp", bufs=2, space=bass.MemorySpace.PSUM) as stp,
            tc.tile_pool(name="tpp", bufs=2, space=bass.MemorySpace.PSUM) as tpp,
            tc.tile_pool(name="trawp", bufs=1, space=bass.MemorySpace.PSUM) as trawp,
            tc.tile_pool(name="abp", bufs=3, space=bass.MemorySpace.PSUM) as abp,
        ):
            # batch 0's critical-path inputs go first: qq block 0, then
            # (inside ph0(0)) cb0/cT0; ident/cmR defer behind them
            qq_all = const.tile([128, BPC * QB], BF16, tag="qq_all")
            nc.sync.dma_start(
                qq_all[:, 0:2 * QL],
                qq_ext[:, 0:2 * QL],
            )
            ident = const.tile([128, 128], BF16, tag="ident")
            cmR = const.tile([128, BPC * NK + BPC], F32, tag="cmR")

            def _ph0(b, s):
                # input DMAs
                s["qmod"] = qq_all[:, b * QB:b * QB + QL]
                s["qts"] = qq_all[:, b * QB + QL:b * QB + QB]  # [q | Ts]
                s["ts_dst"] = qq_all[:, b * QB + 2 * QL:b * QB + QB]
                # cb: 8 groups of [128 ctx cols | ones col], host-swizzled so
                # each partition line is contiguous in DRAM
                cb_t = cbuf.tile([128, NK * 129], BF16, tag="cb")
                nc.sync.dma_start(cb_t[:], cb_ext[b])
                cT_t = ctbuf.tile([128, CL], BF16, tag="cT")
                nc.sync.dma_start(cT_t[:], cT_ext[b])
                if b == 0:
                    nc.sync.dma_start(cmR[:], cmR_ext[:])
                    nc.sync.dma_start(ident[:], id_ext[:])
                    nc.sync.dma_start(
                        qq_all[:].rearrange("p (b t) -> p b t", t=QB)[:, 1:, 0:2 * QL],
                        qq_ext[:].rearrange("p (b t) -> p b t", t=2 * QL)[:, 1:, :],
                    )
                s["cb_t"] = cb_t
                s["cT_t"] = cT_t

            def _ph1(b, s):
                # S^T = qmodT.T @ cT ; E0T = exp(S^T + R[j])  [j, 1024] bf16
                s["rowsum"] = rsp.tile([128, NK], F32, tag="rowsum", name="rowsum")
                s["rcprow"] = rsp.tile([128, NK], F32, tag="rcprow", name="rcprow")
                s["rcp2"] = rsp.tile([128, 1], F32, tag="rcp2", name="rcp2")
                e0t_t = e0tp.tile([128, CL], BF16, tag="e0t")
                for h in range(2):
                    sp = stp.tile([128, 512], F32, tag="sp")
                    nc.tensor.matmul(
                        sp[:], s["qmod"], s["cT_t"][:, h * 512:(h + 1) * 512]
                    )
                    nc.scalar.activation(
                        e0t_t[:, h * 512:(h + 1) * 512], sp[:], AF.Exp,
                        bias=cmR[:, BPC * NK + b:BPC * NK + b + 1],
                    )
                s["e0t_t"] = e0t_t

            def _ph2(b, s):
                # per chunk: E0 natural (bf16) + rowsum via DVE accum
                e0_l = []
                for k in range(NK):
                    ep = tpp.tile([128, 128], BF16, tag="ep")
                    nc.tensor.transpose(
                        ep[:], s["e0t_t"][:, k * 128:(k + 1) * 128], ident[:]
                    )
                    e0_t = e0p.tile([128, 128], BF16, tag="e0")
                    nc.vector.tensor_scalar(
                        e0_t[:], ep[:], 1.0, 0.0, ALU.mult, ALU.add,
                        accum_out=s["rowsum"][:, k:k + 1],
                    )
                    e0_l.append(e0_t)
                    # per-pair reciprocal: G of chunks k-1,k can start before
                    # the remaining chunks' rowsums land
                    if k % 2 == 1:
                        nc.vector.reciprocal(
                            s["rcprow"][:, k - 1:k + 1],
                            s["rowsum"][:, k - 1:k + 1],
                        )
                s["e0_l"] = e0_l

            def _ph3(b, s):
                # G = exp(E0*rcprow + cmb) ; Traw accum (col 128 = colsum)
                traw = trawp.tile([128, 129], F32, tag="traw")
                for k in range(NK):
                    col = b * NK + k
                    g_t = gp.tile([128, QL], BF16, tag="g")
                    nc.scalar.activation(
                        g_t[:], s["e0_l"][k][:], AF.Exp,
                        bias=cmR[:, col:col + 1],
                        scale=s["rcprow"][:, k:k + 1],
                    )
                    nc.tensor.matmul(
                        traw[:, 0:129], g_t[:],
                        s["cb_t"][:, k * 129:(k + 1) * 129],
                        start=(k == 0), stop=(k == NK - 1),
                    )
                nc.vector.reciprocal(s["rcp2"][:], traw[:, 128:129])
                nc.scalar.activation(s["ts_dst"], traw[:, 0:128], AF.Copy, scale=s["rcp2"][:])

            def _ph4(b, s, k2):
                # [Araw|Braw] = E0T_k.T @ [q | Ts] ; stage 2 chunks per DMA
                rcprow = s["rcprow"]
                outg = out_ext[b].rearrange("(g p) d -> p g d", p=128)
                cbg = s["cb_t"][:].rearrange("p (k d) -> p k d", d=129)
                if True:
                    st = stg.tile([128, 8 * D], F32, tag="st")
                    stg4 = st[:].rearrange("p (g d) -> p g d", d=128)
                    # c cols of both chunks in one gpsimd op (bf16 -> f32)
                    nc.gpsimd.tensor_copy(
                        st[:].rearrange("p (g d) -> p g d", d=512)[:, :, 0:128],
                        cbg[:, 2 * k2:2 * k2 + 2, 0:128],
                    )
                    for j in range(2):
                        k = 2 * k2 + j
                        off = j * 4 * D
                        ab = abp.tile([128, 2 * QL], F32, tag="ab")
                        nc.tensor.matmul(
                            ab[:], s["e0t_t"][:, k * 128:(k + 1) * 128], s["qts"]
                        )
                        # col 128:256 = A = Araw * rcprow (ACT/DVE alternate
                        # by chunk to balance engine load)
                        if k % 2 == 0:
                            nc.scalar.activation(
                                st[:, off + 128:off + 256], ab[:, 0:128],
                                AF.Copy, scale=rcprow[:, k:k + 1],
                            )
                        else:
                            nc.vector.tensor_scalar_mul(
                                st[:, off + 128:off + 256], ab[:, 0:128],
                                rcprow[:, k:k + 1],
                            )
                        # cols 256:512 = [c*A | c*Bm] in one DVE op:
                        # (ab[A|B] * rcprow) * [c | c]  (stride-0 broadcast c)
                        ab2 = ab[:].rearrange("p (g d) -> p g d", d=128)
                        out2 = stg4[:, 4 * j + 2:4 * j + 4, :]
                        cc1 = cbg[:, k:k + 1, 0:128]
                        cc2, _ = bass.broadcast_tensor_aps(cc1, ab2)
                        nc.vector.scalar_tensor_tensor(
                            out2, ab2, rcprow[:, k:k + 1],
                            cc2, ALU.mult, ALU.mult,
                        )
                    nc.sync.dma_start(outg[:, 2 * k2:2 * k2 + 2, :], st[:])

            def _mk4(k2):
                return lambda b, s: _ph4(b, s, k2)

            PHASES = [_ph0, _ph1, _ph2, _ph3] + [_mk4(k2) for k2 in range(4)]

            def _emit_all():
                states = [dict() for _ in range(BPC)]
                if SKEW == 0:
                    for b in range(BPC):
                        for ph in PHASES:
                            ph(b, states[b])
                else:
                    # software-pipelined: batch b runs phase p at step
                    # b*SKEW + OFF[p]
                    last = (BPC - 1) * SKEW + max(OFF)
                    for step in range(last + 1):
                        for b in range(BPC):
                            for p in range(len(PHASES)):
                                if b * SKEW + OFF[p] == step:
                                    PHASES[p](b, states[b])

            if loop_n:
                with tc.For_i(0, loop_n, 1):
                    _emit_all()
            else:
                _emit_all()
    return nc


def _prep(c, q, c_mask, q_mask, c_weight, q_weight, cq_weight, bias):
    c = np.asarray(c, dtype=np.float32)
    q = np.asarray(q, dtype=np.float32)
    c_mask = np.asarray(c_mask)
    q_mask = np.asarray(q_mask)
    q_weight = np.asarray(q_weight, dtype=np.float32)
    cq_weight = np.asarray(cq_weight, dtype=np.float32)

    # host-side prep (tiny). NOTE: c@c_weight and bias cancel in softmax_j.
    s1 = (q.reshape(-1, D) @ q_weight).reshape(B, QL)          # (B, 128)
    R = s1 + np.where(q_mask > 0, 0.0, MASK_NEG).astype(np.float32)
    cmb = np.where(c_mask > 0, 0.0, MASK_NEG).astype(np.float32)  # (B, 1024)

    # cb swizzle: [B, p, k*129 + d] = [c[b, k*128+p, d] | 1.0], bf16
    cb = np.ones((B, 128, NK, 129), dtype=ml_dtypes.bfloat16)
    cb[:, :, :, 0:128] = (
        c.reshape(B, NK, 128, D).transpose(0, 2, 1, 3).astype(ml_dtypes.bfloat16)
    )
    cb = cb.reshape(B, 128, NK * 129)

    cT = np.ascontiguousarray(c.transpose(0, 2, 1)).astype(ml_dtypes.bfloat16)
    qmodT = np.ascontiguousarray(
        (q * cq_weight.reshape(1, 1, D)).transpose(0, 2, 1)
    ).astype(ml_dtypes.bfloat16)
    qT_rows = q.astype(ml_dtypes.bfloat16)                     # (B, 128, 128) [j, e]
    qq = np.concatenate([qmodT, qT_rows], axis=2)              # (B, 128, 256)

    in_maps = []
    for core in range(N_CORES):
        sl = slice(core * BPC, (core + 1) * BPC)
        cmT = cmb[sl].reshape(BPC, NK, 128).transpose(2, 0, 1).reshape(128, BPC * NK)
        cmR = np.ascontiguousarray(
            np.concatenate([cmT, R[sl].T], axis=1)             # (128, 64+8)
        )
        qq_core = np.ascontiguousarray(
            qq[sl].transpose(1, 0, 2).reshape(128, BPC * 2 * QL)
        )
        in_maps.append({
            "cb": np.ascontiguousarray(cb[sl]),
            "cT": cT[sl],
            "qq": qq_core,
            "cmR": cmR,
            "ident": np.eye(128, dtype=ml_dtypes.bfloat16),
        })
    return in_maps


def make_in_maps():
    """For the local test/compare harness only (imports reference)."""
    import reference
    inputs = {k: np.asarray(v) for k, v in reference.setup_inputs().items()}
    return _prep(**inputs)


def kernel(c, q, c_mask, q_mask, c_weight, q_weight, cq_weight, bias):
    global LAST_RESULTS
    in_maps = _prep(c, q, c_mask, q_mask, c_weight, q_weight, cq_weight, bias)
    os.environ["BASS_NEVER_TRACE"] = "1"  # no NTFF hook in this container
    nc = _build_graph()
    nc.finalize()
    res = run_bass_kernel_spmd(nc, in_maps, core_ids=list(range(N_CORES)))
    LAST_RESULTS = (nc, in_maps)
    return np.concatenate([res.results[i]["out"] for i in range(N_CORES)], axis=0)
